# revision 1
# baseline (speedup 1.0000x reference)
"""Trainium2 Bass kernel for nn_BlockCrossAttn (block-diagonal attention, E=H=1).

Math per (block b, batch n) pair (256-long vectors q', k', v of the block):
    q' = wq*Q + bq ; k' = wk*K + bk
    soft[q,k] = softmax_k(q'[q] * k'[k])
    out[q] = wvo * (sum_k soft[q,k] * V[k]) + (bvo + bo)
where wvo = wo*wv, bvo = wo*bv (the V/out affine folds into the epilogue
because softmax weights sum to 1).  No max-subtraction: |scores| <= ~27
worst case, exp is safe in fp32.

Sharding: 128 blocks of 256 rows; 16 blocks per core across 8 cores
(fully independent, no collectives).

Per-core device pipeline (512 pairs):
  - PE outer products (contraction dim 1) build S^T[k, q] in PSUM,
    3 pairs per 3-bank group, double buffered.
  - ScalarE exp over [128, 1536] PSUM spans -> E in SBUF.
  - PE reduction matmuls: lhsT = [ones, v] 2-column AP (arbitrary free
    stride), rhs = E streams -> PSUM [2, 256] = (denom, numer) rows per
    pair; every matmul start=True/stop=True (no PSUM accumulation -> no
    whole-bank has_written hazards); 4 pairs per result bank.
  - VectorE flushes banks to SBUF; a DRAM scratch bounce re-lays 32 pairs
    into a dense [32, 1024] tile (one writer); VectorE adds the two ktile
    partials, reciprocal_approx_fast + multiply + affine epilogue;
    one contiguous DMA per block to the n-major output.

Weight scalars are baked into the module as immediates (compiled per
weight set, cached) to avoid TensorScalarPtr sync-wait limits.
"""

from contextlib import ExitStack

import numpy as np

import concourse.bacc as bacc
import concourse.bass as bass
import concourse.tile as tile
from concourse import mybir
from concourse.bass_utils import run_bass_kernel_spmd

FP = mybir.dt.float32
AF = mybir.ActivationFunctionType
ALU = mybir.AluOpType

L = 32768          # sequence length
N = 32             # batch
BS = 256           # block size
NB = L // BS       # 128 blocks
NCORES = 8
BPC = NB // NCORES  # 16 blocks per core
LS = BPC * BS       # 4096 rows per core shard

GROUP = 3           # pairs per exp staging group (3 PSUM banks)
PAIRS = BPC * N     # 512 pairs per core
F32R = mybir.dt.float32r
BF16 = mybir.dt.bfloat16
F32R_OUTER = True   # full-rate relaxed-precision fp32 matmul for scores
EDT = BF16          # E dtype for the reductions (rounding cancels in ratio)


def build_kernel_module(sc, reps: int = 1) -> bass.Bass:
    """sc: dict of python-float weight scalars baked as immediates.

    reps > 1 wraps the whole body in a device-side For_i loop — used only
    for wall-clock benchmarking (amplifies device time over dispatch noise).
    """
    nc = bacc.Bacc("TRN2", target_bir_lowering=False, debug=False, num_devices=NCORES)
    # qkt[4n+c, :] = [qT[n, 1024c:1024(c+1)] | kT[n, 1024c:1024(c+1)]]
    qkt = nc.declare_dram_parameter("qkt", [128, 2048], FP, isOutput=False)
    v = nc.declare_dram_parameter("v", [LS, N], FP, isOutput=False)
    out_t = nc.declare_dram_parameter("out_t", [N, LS], FP, isOutput=True)

    with tile.TileContext(nc) as tc:
        with ExitStack() as ctx:
            if reps == 1:
                _emit(ctx, tc, qkt, v, out_t, sc)
            else:
                with tc.For_i(0, reps, 1):
                    _emit(ctx, tc, qkt, v, out_t, sc)
    nc.compile()
    return nc


def _emit(ctx, tc, qkt, v, out_t, sc):
    nc = tc.nc

    rows = ctx.enter_context(tc.tile_pool(name="rows", bufs=1))
    stage = ctx.enter_context(tc.tile_pool(name="stage", bufs=2))
    vpool = ctx.enter_context(tc.tile_pool(name="vpool", bufs=1))
    epool = ctx.enter_context(tc.tile_pool(name="epool", bufs=3))
    dpool = ctx.enter_context(tc.tile_pool(name="dpool", bufs=2))
    ps_stage = ctx.enter_context(tc.tile_pool(name="ps_stage", bufs=2, space="PSUM"))
    ps_res = ctx.enter_context(tc.tile_pool(name="ps_res", bufs=2, space="PSUM"))
    drs = ctx.enter_context(tc.tile_pool(name="drs", bufs=2, space="DRAM"))

    # --- prep ------------------------------------------------------------------
    QKDT = F32R if F32R_OUTER else FP
    qk4 = rows.tile([128, 2048], QKDT, name="qk4", tag="qk4")
    nc.sync.dma_start(out=qk4[:].bitcast(FP), in_=qkt[:])
    nc.vector.tensor_scalar(
        out=qk4[:, 0:1024], in0=qk4[:, 0:1024].bitcast(FP),
        scalar1=sc["wq"], scalar2=sc["bq"], op0=ALU.mult, op1=ALU.add,
    )
    nc.vector.tensor_scalar(
        out=qk4[:, 1024:2048], in0=qk4[:, 1024:2048].bitcast(FP),
        scalar1=sc["wk"], scalar2=sc["bk"], op0=ALU.mult, op1=ALU.add,
    )

    # [ones, v] tiles: col 0 = 1.0 (memset once); cols 1..64 = raw V of the
    # block, [t, n] order.  Two fixed tiles used alternately per block.
    vcombs = []
    for name in ("vcA", "vcB"):
        vc = vpool.tile([128, 2, N, 3], EDT, name=name, tag=name)
        nc.vector.memset(vc[:], 1.0)
        vcombs.append(vc)

    def load_vcomb(b):
        # DMA raw V, then split into bf16 hi+lo columns (exact to ~2^-16).
        vc = vcombs[b % 2]
        vch = vpool.tile([128, 2, N], FP, name="vch", tag="vch", bufs=2)
        hi32 = vpool.tile([128, 2, N], FP, name="hi32", tag="hi32", bufs=2)
        nc.sync.dma_start(
            out=vch[:],
            in_=v[b * BS:(b + 1) * BS, :].rearrange("(t p) n -> p t n", p=128),
        )
        vc4 = vc[:]
        nc.vector.tensor_copy(vc4[:, :, :, 1], vch[:])
        nc.vector.tensor_copy(hi32[:], vc4[:, :, :, 1])
        nc.vector.tensor_sub(vc4[:, :, :, 2], vch[:], hi32[:])
        return vc

    # --- per-half-block q/k row staging (to partition 0) -----------------------
    def stage_rows(b, h):
        # row n (16h <= n < 16h+16): q at [0, (2(n-16h))*256:...],
        #                            k at [0, (2(n-16h)+1)*256:...]
        qks = stage.tile([1, 16 * 2 * BS], QKDT, name="qks", tag="qks")
        qv = qk4[:].rearrange("(n c) (g f) -> n c g f", c=4, g=2)
        cb, cc = b // 4, (b % 4) * BS
        nc.sync.dma_start(out=qks[:], in_=qv[16 * h:16 * (h + 1), cb, :, cc:cc + BS])
        return qks

    # --- main loop --------------------------------------------------------------
    vcur = [None]
    res_state = {"tile": None, "count": 0, "nflush": 0, "rs": None, "first_g": 0}

    def emit_reduces(pend):
        e, members = pend
        for (s, b, n, vc) in members:
            g = b * N + n
            r = res_state["count"]
            if r == 0:
                res_state["tile"] = ps_res.tile([128, 512], FP, name="res", tag="res")
                if res_state["nflush"] == 0:
                    res_state["rs"] = dpool.tile([128, 4096], FP, name="rs", tag="rs")
                    res_state["first_g"] = g
            jj = r
            for t in (0, 1):
                nc.tensor.matmul(
                    res_state["tile"][32 * jj:32 * jj + 3, t * 256:(t + 1) * 256],
                    lhsT=vc[:][:, t, n, :],
                    rhs=e[:][:, s * 512 + t * 256: s * 512 + (t + 1) * 256],
                    start=True, stop=True,
                    tile_position=(0, 32 * jj),
                )
            res_state["count"] += 1
            if res_state["count"] == 4:
                m = res_state["nflush"]
                nc.vector.tensor_copy(
                    res_state["rs"][:, m * 512:(m + 1) * 512], res_state["tile"][:]
                )
                res_state["count"] = 0
                res_state["tile"] = None
                res_state["nflush"] += 1
                if res_state["nflush"] == 8:
                    division_batch()

    def division_batch():
        b0 = res_state["first_g"] // N
        rs = res_state["rs"]
        # rows {32j+r} of rs -> DRAM scratch already in dense layout:
        # scr[4m+j, r*512 + tq] ; then scratch -> dn is a contiguous copy.
        scr = drs.tile([N, 1536], FP, name="scr", tag="scr")
        rsv = rs[:].rearrange("(j p2) (m tq) -> j p2 m tq", j=4, m=8)
        sw = scr[:].rearrange("(m j) (r tq) -> j m r tq", m=8, r=3)
        for r in (0, 1, 2):
            nc.sync.dma_start(out=sw[:, :, r, :], in_=rsv[:, r, :, :])
        # scratch -> dense [32, 1536]: partition 4m+j (= local pair n), free (r,t,q)
        dn = dpool.tile([N, 1536], FP, name="dn", tag="dn")
        nc.sync.dma_start(out=dn[:], in_=scr[:])
        dnv = dn[:].rearrange("p (r t q) -> p r t q", r=3, t=2)
        den = dpool.tile([N, BS], FP, name="den", tag="den")
        num = dpool.tile([N, BS], FP, name="num", tag="num")
        nc.vector.tensor_add(den[:], dnv[:, 0, 0, :], dnv[:, 0, 1, :])
        nc.vector.tensor_add(num[:], dnv[:, 1, 0, :], dnv[:, 1, 1, :])
        nc.vector.tensor_add(num[:], num[:], dnv[:, 2, 0, :])
        nc.vector.tensor_add(num[:], num[:], dnv[:, 2, 1, :])
        nc.vector.reciprocal_approx_fast(out=den[:], in_=den[:])
        ov = dpool.tile([N, BS], FP, name="ov", tag="ov")
        nc.vector.tensor_mul(ov[:], num[:], den[:])
        nc.vector.tensor_scalar(
            out=ov[:], in0=ov[:], scalar1=sc["wvo"], scalar2=sc["bvo"] + sc["bo"],
            op0=ALU.mult, op1=ALU.add,
        )
        nc.sync.dma_start(out=out_t[:, b0 * BS:(b0 + 1) * BS], in_=ov[:])
        res_state["nflush"] = 0
        res_state["rs"] = None

    pending = None
    cur_stage = None
    cur_rows = None
    members = []
    for g in range(PAIRS):
        b, n = divmod(g, N)
        if n == 0:
            vcur[0] = load_vcomb(b)
        if n % 16 == 0:
            cur_rows = stage_rows(b, n // 16)
        qks = cur_rows
        nn = n % 16
        s = g % GROUP
        if s == 0:
            cur_stage = ps_stage.tile([128, GROUP * 512], FP, name="st", tag="st")
            members = []
        for t in (0, 1):
            lhsT = qks[:][0:1, (2 * nn + 1) * BS + t * 128: (2 * nn + 1) * BS + (t + 1) * 128]
            rhs = qks[:][0:1, (2 * nn) * BS: (2 * nn + 1) * BS]
            nc.tensor.matmul(
                cur_stage[:, s * 512 + t * 256: s * 512 + (t + 1) * 256],
                lhsT=lhsT, rhs=rhs,
                start=True, stop=True,
                tile_position=(0, 0),
            )
        members.append((s, b, n, vcur[0]))
        if s == GROUP - 1 or g == PAIRS - 1:
            e = epool.tile([128, GROUP * 512], EDT, name="e", tag="e")
            width = len(members) * 512
            nc.scalar.activation(e[:][:, 0:width], cur_stage[:][:, 0:width], AF.Exp)
            if pending is not None:
                emit_reduces(pending)
            pending = (e, members)
    emit_reduces(pending)
    assert res_state["count"] == 0 and res_state["nflush"] == 0, (
        "pair count must be a multiple of 32 (one block per division batch)"
    )


_CACHE: dict = {}


def _get_nc(sc, reps: int = 1) -> bass.Bass:
    key = (tuple(sorted(sc.items())), reps)
    if key not in _CACHE:
        _CACHE[key] = build_kernel_module(sc, reps)
    return _CACHE[key]


def make_in_maps(query, key, value, in_proj_w, in_proj_b, out_proj_w, out_proj_b):
    q = np.ascontiguousarray(np.asarray(query, dtype=np.float32).reshape(L, N))
    k = np.ascontiguousarray(np.asarray(key, dtype=np.float32).reshape(L, N))
    vv = np.ascontiguousarray(np.asarray(value, dtype=np.float32).reshape(L, N))
    wq, wk, wv = [float(x) for x in np.asarray(in_proj_w, dtype=np.float32).reshape(3)]
    bq, bk, bv = [float(x) for x in np.asarray(in_proj_b, dtype=np.float32).reshape(3)]
    wo = float(np.asarray(out_proj_w, dtype=np.float32).reshape(1)[0])
    bo = float(np.asarray(out_proj_b, dtype=np.float32).reshape(1)[0])
    sc = {"wq": wq, "bq": bq, "wk": wk, "bk": bk,
          "wvo": float(np.float32(wo) * np.float32(wv)),
          "bvo": float(np.float32(wo) * np.float32(bv)), "bo": bo}
    in_maps = []
    for c in range(NCORES):
        sl = slice(c * LS, (c + 1) * LS)
        qr = np.ascontiguousarray(q[sl].T).reshape(N, 4, LS // 4)
        kr = np.ascontiguousarray(k[sl].T).reshape(N, 4, LS // 4)
        qkt_np = np.concatenate([qr, kr], axis=2).reshape(128, 2048)
        in_maps.append({
            "qkt": np.ascontiguousarray(qkt_np),
            "v": np.ascontiguousarray(vv[sl]),
        })
    return in_maps, sc


def run(in_maps, sc, **kwargs):
    return run_bass_kernel_spmd(_get_nc(sc), in_maps, list(range(NCORES)), **kwargs)


def assemble(results) -> np.ndarray:
    outs = [np.asarray(results[c]["out_t"], dtype=np.float32).T for c in range(NCORES)]
    return np.ascontiguousarray(np.concatenate(outs, axis=0)).reshape(L, N, 1)


def kernel(query, key, value, in_proj_w, in_proj_b, out_proj_w, out_proj_b):
    in_maps, sc = make_in_maps(
        query, key, value, in_proj_w, in_proj_b, out_proj_w, out_proj_b
    )
    res = run(in_maps, sc)
    return assemble(res.results)



# revision 10
# speedup vs baseline: 1.3717x; 1.3717x over previous
"""Trainium2 Bass kernel for nn_BlockCrossAttn (block-diagonal attention, E=H=1).

Math per (block b, batch n) pair (256-long vectors q', k', v' of the block):
    q' = wq*Q + bq ; k' = wk*K + bk      (folded on host)
    soft[q,k] = softmax_k(q'[q] * k'[k])
    out[q] = sum_k soft[q,k] * (wo*wv*V[k]) + (wo*bv + bo)   (bias added on host)
No max-subtraction: |scores| <= ~27 worst case, exp is safe in fp32.

Sharding: 128 blocks of 256 rows; 16 blocks per core across 8 cores
(fully independent, no collectives).

Per-core device pipeline (512 pairs):
  - Scores: per pair ONE f32r matmul: lhsT [2, 128] = both 128-long k-halves
    (one weight load covers the pair), rhs [2, 512] = zero-padded q rows
    ([q|0] on row 0, [0|q] on row 1) -> S^T[k-in-half, (t, q)] = [128, 512]
    in PSUM.  Pairs rotate tile_position rows {0,32,64,96} so weight loads
    overlap matmuls of other strips.
  - ScalarE exp over [128, 1536] PSUM spans (3 pairs/group) -> E bf16 SBUF.
  - Reduce: lhsT = E 128-col slices (bf16 -> fast weight load), rhs =
    [ones, v_hi, v_lo] 3 columns -> PSUM [q-half 128, 3] = (den, num_hi,
    num_lo) per (pair, q-half, k-half), all partition-ALIGNED per q.
    Every matmul start=stop=True (no accumulation hazards).
  - VectorE epilogue straight on the result bank: den = t0+t1, num = 4-way
    add, reciprocal_approx_fast, multiply; contiguous DMA to a [128, 1024]
    q-major output that the host unscrambles.

All weight scalars are folded into the inputs on the host, so the compiled
module is weight-independent (compiled once, cached).
"""

from contextlib import ExitStack

import numpy as np

import concourse.bacc as bacc
import concourse.bass as bass
import concourse.tile as tile
from concourse import mybir
from concourse.bass_utils import run_bass_kernel_spmd

FP = mybir.dt.float32
F32R = mybir.dt.float32r
BF16 = mybir.dt.bfloat16
U16 = mybir.dt.uint16
AF = mybir.ActivationFunctionType

L = 32768          # sequence length
N = 32             # batch
BS = 256           # block size
NB = L // BS       # 128 blocks
NCORES = 8
BPC = NB // NCORES  # 16 blocks per core
LS = BPC * BS       # 4096 rows per core shard
PAIRS = BPC * N     # 512 (block, batch) pairs per core

GROUP = 3           # pairs per exp staging group (3 PSUM banks)
STRIPS = 4          # tile_position row strips used for outer matmuls
CHUNK = 16 * STRIPS  # pairs per q/k input chunk
NCH = PAIRS // CHUNK
FLUSH = 42          # pairs per result-bank flush (42*12 = 504 <= 512 cols)


def build_kernel_module(reps: int = 1) -> bass.Bass:
    nc = bacc.Bacc("TRN2", target_bir_lowering=False, debug=False, num_devices=NCORES)
    # q rhs blocks: [ch*2*STRIPS + s*2 + r, slot*512 + half*256 + :256]
    qd = nc.declare_dram_parameter("qd", [NCH * 2 * STRIPS, 8192], F32R, isOutput=False)
    # k lhsT blocks: [ch*2*STRIPS + s*2 + r, slot*128 + :128]
    kd = nc.declare_dram_parameter("kd", [NCH * 2 * STRIPS, 2048], F32R, isOutput=False)
    # v reduce rhs: [p, g*6 + t*3 + (1.0, v_hi, v_lo)] as bf16 bits
    vd = nc.declare_dram_parameter("vd", [128, 3072], U16, isOutput=False)
    # out: [q-in-half, pair*2 + h]
    od = nc.declare_dram_parameter("od", [128, 1024], FP, isOutput=True)

    with tile.TileContext(nc) as tc:
        with ExitStack() as ctx:
            if reps == 1:
                _emit(ctx, tc, qd, kd, vd, od)
            else:
                with tc.For_i(0, reps, 1):
                    _emit(ctx, tc, qd, kd, vd, od)
    nc.compile()
    return nc


def _emit(ctx, tc, qd, kd, vd, od):
    nc = tc.nc

    qpool = ctx.enter_context(tc.tile_pool(name="qpool", bufs=2))
    kpool = ctx.enter_context(tc.tile_pool(name="kpool", bufs=2))
    vpool = ctx.enter_context(tc.tile_pool(name="vpool", bufs=1))
    epool = ctx.enter_context(tc.tile_pool(name="epool", bufs=3))
    dpool = ctx.enter_context(tc.tile_pool(name="dpool", bufs=2))
    ps_stage = ctx.enter_context(tc.tile_pool(name="ps_stage", bufs=2, space="PSUM"))
    ps_res = ctx.enter_context(tc.tile_pool(name="ps_res", bufs=2, space="PSUM"))

    vt = vpool.tile([128, 512, 2, 3], U16, name="vt", tag="vt")
    nc.sync.dma_start(out=vt[:].rearrange("p g t c -> p (g t c)"), in_=vd[:])

    def load_chunk(ch):
        qt = qpool.tile([128, 8192], F32R, name="qt", tag="qt")
        kt = kpool.tile([128, 2048], F32R, name="kt", tag="kt")
        for s in range(STRIPS):
            row = ch * 2 * STRIPS + s * 2
            nc.sync.dma_start(out=qt[32 * s:32 * s + 2, :], in_=qd[row:row + 2, :])
            nc.sync.dma_start(out=kt[32 * s:32 * s + 2, :], in_=kd[row:row + 2, :])
        return qt, kt

    # --- result-bank flush: softmax division epilogue ------------------------
    fs = {"tile": None, "count": 0, "first_g": 0}

    def division_flush():
        nf = fs["count"]
        p0 = fs["first_g"]
        sb = dpool.tile([128, FLUSH * 12], FP, name="sb", tag="sb")
        nc.vector.tensor_copy(sb[:, 0:nf * 12], fs["tile"][:, 0:nf * 12])
        r = sb[:, 0:nf * 12].rearrange("p (i h t c) -> p i h t c", h=2, t=2, c=3)
        den = dpool.tile([128, FLUSH, 2], FP, name="den", tag="den")
        num = dpool.tile([128, FLUSH, 2], FP, name="num", tag="num")
        na = dpool.tile([128, FLUSH, 2], FP, name="na", tag="na")
        rcp = dpool.tile([128, FLUSH, 2], FP, name="rcp", tag="rcp")
        dn, nm, nA, rc = den[:, 0:nf, :], num[:, 0:nf, :], na[:, 0:nf, :], rcp[:, 0:nf, :]
        nc.vector.tensor_add(dn, r[:, :, :, 0, 0], r[:, :, :, 1, 0])
        nc.vector.tensor_add(nA, r[:, :, :, 0, 1], r[:, :, :, 0, 2])
        nc.vector.tensor_add(nm, r[:, :, :, 1, 1], r[:, :, :, 1, 2])
        nc.vector.tensor_add(nm, nm, nA)
        nc.vector.reciprocal_approx_fast(out=rc, in_=dn)
        nc.vector.tensor_mul(nm, nm, rc)
        nc.sync.dma_start(
            out=od[:, p0 * 2:p0 * 2 + nf * 2],
            in_=nm.rearrange("p i h -> p (i h)"),
        )
        fs["tile"] = None
        fs["count"] = 0

    def emit_reduces(pend):
        e, members = pend
        for (s, g) in members:
            if fs["count"] == 0:
                fs["tile"] = ps_res.tile([128, 512], FP, name="res", tag="res")
                fs["first_g"] = g
            col0 = fs["count"] * 12
            for h in (0, 1):
                for t in (0, 1):
                    nc.tensor.matmul(
                        fs["tile"][:, col0 + h * 6 + t * 3: col0 + h * 6 + t * 3 + 3],
                        lhsT=e[:][:, s * 512 + t * 256 + h * 128: s * 512 + t * 256 + (h + 1) * 128],
                        rhs=vt[:, g, t, :].bitcast(BF16),
                        start=True, stop=True,
                        tile_position=(0, 0),
                    )
            fs["count"] += 1
            if fs["count"] == FLUSH or g == PAIRS - 1:
                division_flush()

    # --- main loop -----------------------------------------------------------
    qt = kt = None
    cur_stage = None
    members = []
    pending = None
    for g in range(PAIRS):
        if g % CHUNK == 0:
            qt, kt = load_chunk(g // CHUNK)
        j = g % CHUNK
        strip, slot = j % STRIPS, j // STRIPS
        s = g % GROUP
        if s == 0:
            cur_stage = ps_stage.tile([128, GROUP * 512], FP, name="st", tag="st")
            members = []
        nc.tensor.matmul(
            cur_stage[:, s * 512:(s + 1) * 512],
            lhsT=kt[32 * strip:32 * strip + 2, slot * 128:(slot + 1) * 128],
            rhs=qt[32 * strip:32 * strip + 2, slot * 512:(slot + 1) * 512],
            start=True, stop=True,
            tile_position=(32 * strip, 0),
        )
        members.append((s, g))
        if s == GROUP - 1 or g == PAIRS - 1:
            e = epool.tile([128, GROUP * 512], BF16, name="e", tag="e")
            width = len(members) * 512
            nc.scalar.activation(e[:][:, 0:width], cur_stage[:][:, 0:width], AF.Exp)
            if pending is not None:
                emit_reduces(pending)
            pending = (e, members)
    emit_reduces(pending)
    assert fs["count"] == 0


_CACHE: dict = {}


def _get_nc(reps: int = 1) -> bass.Bass:
    if reps not in _CACHE:
        _CACHE[reps] = build_kernel_module(reps)
    return _CACHE[reps]


def _to_bf16_bits(x: np.ndarray) -> np.ndarray:
    """fp32 -> bf16 bit pattern (round-to-nearest-even), as uint16."""
    u = x.astype(np.float32).view(np.uint32)
    rounded = u + 0x7FFF + ((u >> 16) & 1)
    return (rounded >> 16).astype(np.uint16)


def _bf16_to_f32(bits: np.ndarray) -> np.ndarray:
    return (bits.astype(np.uint32) << 16).view(np.float32)


def make_in_maps(query, key, value, in_proj_w, in_proj_b, out_proj_w, out_proj_b):
    q = np.asarray(query, dtype=np.float32).reshape(L, N)
    k = np.asarray(key, dtype=np.float32).reshape(L, N)
    v = np.asarray(value, dtype=np.float32).reshape(L, N)
    wq, wk, wv = [float(x) for x in np.asarray(in_proj_w, dtype=np.float32).reshape(3)]
    bq, bk, bv = [float(x) for x in np.asarray(in_proj_b, dtype=np.float32).reshape(3)]
    wo = float(np.asarray(out_proj_w, dtype=np.float32).reshape(1)[0])
    bo = float(np.asarray(out_proj_b, dtype=np.float32).reshape(1)[0])

    qp = q * np.float32(wq) + np.float32(bq)
    kp = k * np.float32(wk) + np.float32(bk)
    vp = v * (np.float32(wo) * np.float32(wv))
    out_bias = float(np.float32(wo) * np.float32(bv) + np.float32(bo))

    one_bits = np.uint16(0x3F80)  # bf16 1.0
    in_maps = []
    for c in range(NCORES):
        sl = slice(c * LS, (c + 1) * LS)
        # [g, 256] with g = b*32 + n
        Q = np.ascontiguousarray(qp[sl].reshape(BPC, BS, N).transpose(0, 2, 1)).reshape(PAIRS, BS)
        K = np.ascontiguousarray(kp[sl].reshape(BPC, BS, N).transpose(0, 2, 1)).reshape(PAIRS, BS)
        V = np.ascontiguousarray(vp[sl].reshape(BPC, BS, N).transpose(0, 2, 1)).reshape(PAIRS, BS)

        # q rhs: [ch, s, r, slot, half, 256] -> [NCH*2*STRIPS, 8192]
        Qv = Q.reshape(NCH, 16, STRIPS, BS)               # [ch, slot, s, :]
        q4 = np.zeros((NCH, STRIPS, 2, 16, 2, BS), np.float32)
        q4[:, :, 0, :, 0, :] = Qv.transpose(0, 2, 1, 3)
        q4[:, :, 1, :, 1, :] = Qv.transpose(0, 2, 1, 3)
        qd_np = q4.reshape(NCH * 2 * STRIPS, 8192)

        # k lhsT: [ch, s, r, slot, 128] -> [NCH*2*STRIPS, 2048]
        Kv = K.reshape(NCH, 16, STRIPS, 2, 128)           # [ch, slot, s, r, :]
        kd_np = np.ascontiguousarray(Kv.transpose(0, 2, 3, 1, 4)).reshape(NCH * 2 * STRIPS, 2048)

        # v reduce rhs: [p, g, t, (1, hi, lo)] -> [128, 3072] bf16 bits
        vhi_bits = _to_bf16_bits(V)
        vlo_bits = _to_bf16_bits(V - _bf16_to_f32(vhi_bits))
        vr = np.empty((128, PAIRS, 2, 3), np.uint16)
        vr[:, :, :, 0] = one_bits
        vr[:, :, :, 1] = vhi_bits.reshape(PAIRS, 2, 128).transpose(2, 0, 1)
        vr[:, :, :, 2] = vlo_bits.reshape(PAIRS, 2, 128).transpose(2, 0, 1)

        in_maps.append({
            "qd": np.ascontiguousarray(qd_np),
            "kd": kd_np,
            "vd": np.ascontiguousarray(vr.reshape(128, 3072)),
        })
    return in_maps, out_bias


def run(in_maps, **kwargs):
    return run_bass_kernel_spmd(_get_nc(), in_maps, list(range(NCORES)), **kwargs)


def assemble(results, out_bias) -> np.ndarray:
    shards = []
    for c in range(NCORES):
        od = np.asarray(results[c]["od"], dtype=np.float32)  # [128, 1024]
        arr = od.reshape(128, BPC, N, 2)                     # [qh, b, n, h]
        shards.append(arr.transpose(1, 3, 0, 2).reshape(LS, N))
    out = np.concatenate(shards, axis=0) + np.float32(out_bias)
    return np.ascontiguousarray(out).reshape(L, N, 1)


def kernel(query, key, value, in_proj_w, in_proj_b, out_proj_w, out_proj_b):
    in_maps, out_bias = make_in_maps(
        query, key, value, in_proj_w, in_proj_b, out_proj_w, out_proj_b
    )
    res = run(in_maps)
    return assemble(res.results, out_bias)


# revision 12
# speedup vs baseline: 1.3746x; 1.0021x over previous
"""Trainium2 Bass kernel for nn_BlockCrossAttn (block-diagonal attention, E=H=1).

Math per (block b, batch n) pair (256-long vectors q', k', v' of the block):
    q' = wq*Q + bq ; k' = wk*K + bk      (folded on host)
    soft[q,k] = softmax_k(q'[q] * k'[k])
    out[q] = sum_k soft[q,k] * (wo*wv*V[k]) + (wo*bv + bo)   (bias added on host)
No max-subtraction: |scores| <= ~27 worst case, exp is safe in fp32.

Sharding: 128 blocks of 256 rows; 16 blocks per core across 8 cores
(fully independent, no collectives).

Per-core device pipeline (512 pairs):
  - Scores: per pair ONE f32r matmul: lhsT [2, 128] = both 128-long k-halves
    (one weight load covers the pair), rhs [2, 512] = zero-padded q rows
    ([q|0] on row 0, [0|q] on row 1) -> S^T[k-in-half, (t, q)] = [128, 512]
    in PSUM.  Pairs rotate tile_position rows {0,32,64,96} so weight loads
    overlap matmuls of other strips.
  - ScalarE exp over [128, 1536] PSUM spans (3 pairs/group) -> E bf16 SBUF.
  - Reduce: lhsT = E 128-col slices (bf16 -> fast weight load), rhs =
    [ones, v_hi, v_lo] 3 columns -> PSUM [q-half 128, 3] = (den, num_hi,
    num_lo) per (pair, q-half, k-half), all partition-ALIGNED per q.
    Every matmul start=stop=True (no accumulation hazards).
  - VectorE epilogue straight on the result bank: den = t0+t1, num = 4-way
    add, reciprocal_approx_fast, multiply; contiguous DMA to a [128, 1024]
    q-major output that the host unscrambles.

All weight scalars are folded into the inputs on the host, so the compiled
module is weight-independent (compiled once, cached).
"""

from contextlib import ExitStack

import numpy as np

import concourse.bacc as bacc
import concourse.bass as bass
import concourse.tile as tile
from concourse import mybir
from concourse.bass_utils import run_bass_kernel_spmd

FP = mybir.dt.float32
F32R = mybir.dt.float32r
BF16 = mybir.dt.bfloat16
U16 = mybir.dt.uint16
AF = mybir.ActivationFunctionType

L = 32768          # sequence length
N = 32             # batch
BS = 256           # block size
NB = L // BS       # 128 blocks
NCORES = 8
BPC = NB // NCORES  # 16 blocks per core
LS = BPC * BS       # 4096 rows per core shard
PAIRS = BPC * N     # 512 (block, batch) pairs per core

GROUP = 3           # pairs per exp staging group (3 PSUM banks)
STRIPS = 4          # tile_position row strips used for outer matmuls
CHUNK = 16 * STRIPS  # pairs per q/k input chunk
NCH = PAIRS // CHUNK
FLUSH = 42          # pairs per result-bank flush (42*12 = 504 <= 512 cols)


def build_kernel_module(reps: int = 1) -> bass.Bass:
    nc = bacc.Bacc("TRN2", target_bir_lowering=False, debug=False, num_devices=NCORES)
    # q rhs blocks: [ch*2*STRIPS + s*2 + r, slot*512 + half*256 + :256]
    qd = nc.declare_dram_parameter("qd", [NCH * 2 * STRIPS, 8192], F32R, isOutput=False)
    # k lhsT blocks: [ch*2*STRIPS + s*2 + r, slot*128 + :128]
    kd = nc.declare_dram_parameter("kd", [NCH * 2 * STRIPS, 2048], F32R, isOutput=False)
    # v reduce rhs: [p, g*6 + t*3 + (1.0, v_hi, v_lo)] as bf16 bits
    vd = nc.declare_dram_parameter("vd", [128, 3072], U16, isOutput=False)
    # out: [q-in-half, pair*2 + h]
    od = nc.declare_dram_parameter("od", [128, 1024], FP, isOutput=True)

    with tile.TileContext(nc) as tc:
        with ExitStack() as ctx:
            if reps == 1:
                _emit(ctx, tc, qd, kd, vd, od)
            else:
                with tc.For_i(0, reps, 1):
                    _emit(ctx, tc, qd, kd, vd, od)
    nc.compile()
    return nc


def _emit(ctx, tc, qd, kd, vd, od):
    nc = tc.nc

    qpool = ctx.enter_context(tc.tile_pool(name="qpool", bufs=3))
    kpool = ctx.enter_context(tc.tile_pool(name="kpool", bufs=3))
    vpool = ctx.enter_context(tc.tile_pool(name="vpool", bufs=1))
    epool = ctx.enter_context(tc.tile_pool(name="epool", bufs=3))
    dpool = ctx.enter_context(tc.tile_pool(name="dpool", bufs=2))
    ps_stage = ctx.enter_context(tc.tile_pool(name="ps_stage", bufs=2, space="PSUM"))
    ps_res = ctx.enter_context(tc.tile_pool(name="ps_res", bufs=2, space="PSUM"))

    # Warm the exp table set (~2.7us) while the first DMAs run.
    warm = vpool.tile([1, 8], FP, name="warm", tag="warm")
    nc.vector.memset(warm[:], 0.0)
    nc.scalar.activation(warm[:], warm[:], AF.Exp)

    def load_chunk(ch):
        qt = qpool.tile([128, 8192], F32R, name="qt", tag="qt")
        kt = kpool.tile([128, 2048], F32R, name="kt", tag="kt")
        for s in range(STRIPS):
            row = ch * 2 * STRIPS + s * 2
            nc.sync.dma_start(out=qt[32 * s:32 * s + 2, :], in_=qd[row:row + 2, :])
            nc.sync.dma_start(out=kt[32 * s:32 * s + 2, :], in_=kd[row:row + 2, :])
        return qt, kt

    # --- result-bank flush: softmax division epilogue ------------------------
    fs = {"tile": None, "count": 0, "first_g": 0}

    def division_flush():
        nf = fs["count"]
        p0 = fs["first_g"]
        sb = dpool.tile([128, FLUSH * 12], FP, name="sb", tag="sb")
        nc.vector.tensor_copy(sb[:, 0:nf * 12], fs["tile"][:, 0:nf * 12])
        r = sb[:, 0:nf * 12].rearrange("p (i h t c) -> p i h t c", h=2, t=2, c=3)
        den = dpool.tile([128, FLUSH, 2], FP, name="den", tag="den")
        num = dpool.tile([128, FLUSH, 2], FP, name="num", tag="num")
        na = dpool.tile([128, FLUSH, 2], FP, name="na", tag="na")
        rcp = dpool.tile([128, FLUSH, 2], FP, name="rcp", tag="rcp")
        dn, nm, nA, rc = den[:, 0:nf, :], num[:, 0:nf, :], na[:, 0:nf, :], rcp[:, 0:nf, :]
        nc.vector.tensor_add(dn, r[:, :, :, 0, 0], r[:, :, :, 1, 0])
        nc.vector.tensor_add(nA, r[:, :, :, 0, 1], r[:, :, :, 0, 2])
        nc.vector.tensor_add(nm, r[:, :, :, 1, 1], r[:, :, :, 1, 2])
        nc.vector.tensor_add(nm, nm, nA)
        nc.vector.reciprocal_approx_fast(out=rc, in_=dn)
        nc.vector.tensor_mul(nm, nm, rc)
        nc.sync.dma_start(
            out=od[:, p0 * 2:p0 * 2 + nf * 2],
            in_=nm.rearrange("p i h -> p (i h)"),
        )
        fs["tile"] = None
        fs["count"] = 0

    def emit_reduces(pend):
        e, members = pend
        for (s, g) in members:
            if fs["count"] == 0:
                fs["tile"] = ps_res.tile([128, 512], FP, name="res", tag="res")
                fs["first_g"] = g
            col0 = fs["count"] * 12
            for h in (0, 1):
                for t in (0, 1):
                    nc.tensor.matmul(
                        fs["tile"][:, col0 + h * 6 + t * 3: col0 + h * 6 + t * 3 + 3],
                        lhsT=e[:][:, s * 512 + t * 256 + h * 128: s * 512 + t * 256 + (h + 1) * 128],
                        rhs=vt[:, g, t, :].bitcast(BF16),
                        start=True, stop=True,
                        tile_position=(0, 0),
                    )
            fs["count"] += 1
            if fs["count"] == FLUSH or g == PAIRS - 1:
                division_flush()

    # --- main loop -----------------------------------------------------------
    qt0, kt0 = load_chunk(0)
    vt = vpool.tile([128, 512, 2, 3], U16, name="vt", tag="vt")
    nc.sync.dma_start(out=vt[:].rearrange("p g t c -> p (g t c)"), in_=vd[:])

    qt = kt = None
    cur_stage = None
    members = []
    pending = None
    for g in range(PAIRS):
        if g % CHUNK == 0:
            qt, kt = (qt0, kt0) if g == 0 else load_chunk(g // CHUNK)
        j = g % CHUNK
        strip, slot = j % STRIPS, j // STRIPS
        s = g % GROUP
        if s == 0:
            cur_stage = ps_stage.tile([128, GROUP * 512], FP, name="st", tag="st")
            members = []
        nc.tensor.matmul(
            cur_stage[:, s * 512:(s + 1) * 512],
            lhsT=kt[32 * strip:32 * strip + 2, slot * 128:(slot + 1) * 128],
            rhs=qt[32 * strip:32 * strip + 2, slot * 512:(slot + 1) * 512],
            start=True, stop=True,
            tile_position=(32 * strip, 0),
        )
        members.append((s, g))
        if s == GROUP - 1 or g == PAIRS - 1:
            e = epool.tile([128, GROUP * 512], BF16, name="e", tag="e")
            width = len(members) * 512
            nc.scalar.activation(e[:][:, 0:width], cur_stage[:][:, 0:width], AF.Exp)
            if pending is not None:
                emit_reduces(pending)
            pending = (e, members)
    emit_reduces(pending)
    assert fs["count"] == 0


_CACHE: dict = {}


def _get_nc(reps: int = 1) -> bass.Bass:
    if reps not in _CACHE:
        _CACHE[reps] = build_kernel_module(reps)
    return _CACHE[reps]


def _to_bf16_bits(x: np.ndarray) -> np.ndarray:
    """fp32 -> bf16 bit pattern (round-to-nearest-even), as uint16."""
    u = x.astype(np.float32).view(np.uint32)
    rounded = u + 0x7FFF + ((u >> 16) & 1)
    return (rounded >> 16).astype(np.uint16)


def _bf16_to_f32(bits: np.ndarray) -> np.ndarray:
    return (bits.astype(np.uint32) << 16).view(np.float32)


def make_in_maps(query, key, value, in_proj_w, in_proj_b, out_proj_w, out_proj_b):
    q = np.asarray(query, dtype=np.float32).reshape(L, N)
    k = np.asarray(key, dtype=np.float32).reshape(L, N)
    v = np.asarray(value, dtype=np.float32).reshape(L, N)
    wq, wk, wv = [float(x) for x in np.asarray(in_proj_w, dtype=np.float32).reshape(3)]
    bq, bk, bv = [float(x) for x in np.asarray(in_proj_b, dtype=np.float32).reshape(3)]
    wo = float(np.asarray(out_proj_w, dtype=np.float32).reshape(1)[0])
    bo = float(np.asarray(out_proj_b, dtype=np.float32).reshape(1)[0])

    qp = q * np.float32(wq) + np.float32(bq)
    kp = k * np.float32(wk) + np.float32(bk)
    vp = v * (np.float32(wo) * np.float32(wv))
    out_bias = float(np.float32(wo) * np.float32(bv) + np.float32(bo))

    one_bits = np.uint16(0x3F80)  # bf16 1.0
    in_maps = []
    for c in range(NCORES):
        sl = slice(c * LS, (c + 1) * LS)
        # [g, 256] with g = b*32 + n
        Q = np.ascontiguousarray(qp[sl].reshape(BPC, BS, N).transpose(0, 2, 1)).reshape(PAIRS, BS)
        K = np.ascontiguousarray(kp[sl].reshape(BPC, BS, N).transpose(0, 2, 1)).reshape(PAIRS, BS)
        V = np.ascontiguousarray(vp[sl].reshape(BPC, BS, N).transpose(0, 2, 1)).reshape(PAIRS, BS)

        # q rhs: [ch, s, r, slot, half, 256] -> [NCH*2*STRIPS, 8192]
        Qv = Q.reshape(NCH, 16, STRIPS, BS)               # [ch, slot, s, :]
        q4 = np.zeros((NCH, STRIPS, 2, 16, 2, BS), np.float32)
        q4[:, :, 0, :, 0, :] = Qv.transpose(0, 2, 1, 3)
        q4[:, :, 1, :, 1, :] = Qv.transpose(0, 2, 1, 3)
        qd_np = q4.reshape(NCH * 2 * STRIPS, 8192)

        # k lhsT: [ch, s, r, slot, 128] -> [NCH*2*STRIPS, 2048]
        Kv = K.reshape(NCH, 16, STRIPS, 2, 128)           # [ch, slot, s, r, :]
        kd_np = np.ascontiguousarray(Kv.transpose(0, 2, 3, 1, 4)).reshape(NCH * 2 * STRIPS, 2048)

        # v reduce rhs: [p, g, t, (1, hi, lo)] -> [128, 3072] bf16 bits
        vhi_bits = _to_bf16_bits(V)
        vlo_bits = _to_bf16_bits(V - _bf16_to_f32(vhi_bits))
        vr = np.empty((128, PAIRS, 2, 3), np.uint16)
        vr[:, :, :, 0] = one_bits
        vr[:, :, :, 1] = vhi_bits.reshape(PAIRS, 2, 128).transpose(2, 0, 1)
        vr[:, :, :, 2] = vlo_bits.reshape(PAIRS, 2, 128).transpose(2, 0, 1)

        in_maps.append({
            "qd": np.ascontiguousarray(qd_np),
            "kd": kd_np,
            "vd": np.ascontiguousarray(vr.reshape(128, 3072)),
        })
    return in_maps, out_bias


def run(in_maps, **kwargs):
    return run_bass_kernel_spmd(_get_nc(), in_maps, list(range(NCORES)), **kwargs)


def assemble(results, out_bias) -> np.ndarray:
    shards = []
    for c in range(NCORES):
        od = np.asarray(results[c]["od"], dtype=np.float32)  # [128, 1024]
        arr = od.reshape(128, BPC, N, 2)                     # [qh, b, n, h]
        shards.append(arr.transpose(1, 3, 0, 2).reshape(LS, N))
    out = np.concatenate(shards, axis=0) + np.float32(out_bias)
    return np.ascontiguousarray(out).reshape(L, N, 1)


def kernel(query, key, value, in_proj_w, in_proj_b, out_proj_w, out_proj_b):
    in_maps, out_bias = make_in_maps(
        query, key, value, in_proj_w, in_proj_b, out_proj_w, out_proj_b
    )
    res = run(in_maps)
    return assemble(res.results, out_bias)


# revision 18
# speedup vs baseline: 1.4212x; 1.0339x over previous
"""Trainium2 Bass kernel for nn_BlockCrossAttn (block-diagonal attention, E=H=1).

Math per (block b, batch n) pair (256-long vectors q', k', v' of the block):
    q' = wq*Q + bq ; k' = wk*K + bk      (folded on host)
    soft[q,k] = softmax_k(q'[q] * k'[k])
    out[q] = sum_k soft[q,k] * (wo*wv*V[k]) + (wo*bv + bo)   (bias added on host)
No max-subtraction: |scores| <= ~27 worst case, exp is safe in fp32.

Sharding: 128 blocks of 256 rows; 16 blocks per core across 8 cores
(fully independent, no collectives).

Per-core device pipeline (512 pairs):
  - Scores: per pair ONE f32r matmul: lhsT [2, 128] = both 128-long k-halves
    (one weight load covers the pair), rhs [2, 512] = zero-padded q rows
    ([q|0] on row 0, [0|q] on row 1) -> S^T[k-in-half, (t, q)] = [128, 512]
    in PSUM.  Pairs rotate tile_position rows {0,32,64,96} so weight loads
    overlap matmuls of other strips.
  - ScalarE exp over [128, 1536] PSUM spans (3 pairs/group) -> E bf16 SBUF.
  - Reduce: lhsT = E 128-col slices (bf16 -> fast weight load), rhs =
    [ones, v_hi, v_lo] 3 columns -> PSUM [q-half 128, 3] = (den, num_hi,
    num_lo) per (pair, q-half, k-half), all partition-ALIGNED per q.
    Every matmul start=stop=True (no accumulation hazards).
  - VectorE epilogue straight on the result bank: den = t0+t1, num = 4-way
    add, reciprocal_approx_fast, multiply; contiguous DMA to a [128, 1024]
    q-major output that the host unscrambles.

All weight scalars are folded into the inputs on the host, so the compiled
module is weight-independent (compiled once, cached).
"""

from contextlib import ExitStack

import numpy as np

import concourse.bacc as bacc
import concourse.bass as bass
import concourse.tile as tile
from concourse import mybir
from concourse.bass_utils import run_bass_kernel_spmd

FP = mybir.dt.float32
F32R = mybir.dt.float32r
BF16 = mybir.dt.bfloat16
U16 = mybir.dt.uint16
AF = mybir.ActivationFunctionType

L = 32768          # sequence length
N = 32             # batch
BS = 256           # block size
NB = L // BS       # 128 blocks
NCORES = 8
BPC = NB // NCORES  # 16 blocks per core
LS = BPC * BS       # 4096 rows per core shard
PAIRS = BPC * N     # 512 (block, batch) pairs per core

GROUP = 3           # pairs per exp staging group (3 PSUM banks)
STRIPS = 4          # tile_position row strips used for outer matmuls
CHUNK = 16 * STRIPS  # pairs per full q/k input chunk
# First chunk split small so the first matmuls start ASAP after launch.
CHUNK_SIZES = [8, 24, 32] + [CHUNK] * ((PAIRS - CHUNK) // CHUNK)
assert sum(CHUNK_SIZES) == PAIRS and all(c % STRIPS == 0 for c in CHUNK_SIZES)
CHUNK_STARTS = [sum(CHUNK_SIZES[:i]) for i in range(len(CHUNK_SIZES))]
NCH = len(CHUNK_SIZES)
FLUSH = 42          # pairs per result-bank flush (42*12 = 504 <= 512 cols)


def build_kernel_module(reps: int = 1) -> bass.Bass:
    nc = bacc.Bacc("TRN2", target_bir_lowering=False, debug=False, num_devices=NCORES)
    # q rhs blocks: [ch*2*STRIPS + s*2 + r, slot*512 + half*256 + :256]
    qd = nc.declare_dram_parameter("qd", [NCH * 2 * STRIPS, 8192], F32R, isOutput=False)
    # k lhsT blocks: [ch*2*STRIPS + s*2 + r, slot*128 + :128]
    kd = nc.declare_dram_parameter("kd", [NCH * 2 * STRIPS, 2048], F32R, isOutput=False)
    # v reduce rhs: [p, g*6 + t*3 + (1.0, v_hi, v_lo)] as bf16 bits
    vd = nc.declare_dram_parameter("vd", [128, 3072], U16, isOutput=False)
    # out: [q-in-half, pair*2 + h]
    od = nc.declare_dram_parameter("od", [128, 1024], FP, isOutput=True)

    with tile.TileContext(nc) as tc:
        with ExitStack() as ctx:
            if reps == 1:
                _emit(ctx, tc, qd, kd, vd, od)
            else:
                with tc.For_i(0, reps, 1):
                    _emit(ctx, tc, qd, kd, vd, od)
    nc.compile()
    return nc


def _emit(ctx, tc, qd, kd, vd, od):
    nc = tc.nc

    qpool = ctx.enter_context(tc.tile_pool(name="qpool", bufs=3))
    kpool = ctx.enter_context(tc.tile_pool(name="kpool", bufs=3))
    vpool = ctx.enter_context(tc.tile_pool(name="vpool", bufs=1))
    epool = ctx.enter_context(tc.tile_pool(name="epool", bufs=3))
    dpool = ctx.enter_context(tc.tile_pool(name="dpool", bufs=2))
    ps_stage = ctx.enter_context(tc.tile_pool(name="ps_stage", bufs=2, space="PSUM"))
    ps_res = ctx.enter_context(tc.tile_pool(name="ps_res", bufs=2, space="PSUM"))

    # Warm the exp table set (~2.7us) while the first DMAs run.
    warm = vpool.tile([1, 8], FP, name="warm", tag="warm")
    nc.vector.memset(warm[:], 0.0)
    nc.scalar.activation(warm[:], warm[:], AF.Exp)

    def load_chunk(ci):
        size = CHUNK_SIZES[ci]
        wq = (size // STRIPS) * 512
        wk = (size // STRIPS) * 128
        qt = qpool.tile([128, 8192], F32R, name="qt", tag="qt")
        kt = kpool.tile([128, 2048], F32R, name="kt", tag="kt")
        for s in range(STRIPS):
            row = ci * 2 * STRIPS + s * 2
            nc.sync.dma_start(out=qt[32 * s:32 * s + 2, 0:wq], in_=qd[row:row + 2, 0:wq])
            nc.sync.dma_start(out=kt[32 * s:32 * s + 2, 0:wk], in_=kd[row:row + 2, 0:wk])
        return qt, kt

    # --- result-bank flush: softmax division epilogue ------------------------
    fs = {"tile": None, "count": 0, "first_g": 0}

    def division_flush():
        nf = fs["count"]
        p0 = fs["first_g"]
        sb = dpool.tile([128, FLUSH * 12], FP, name="sb", tag="sb")
        nc.vector.tensor_copy(sb[:, 0:nf * 12], fs["tile"][:, 0:nf * 12])
        r = sb[:, 0:nf * 12].rearrange("p (i h t c) -> p i h t c", h=2, t=2, c=3)
        den = dpool.tile([128, FLUSH, 2], FP, name="den", tag="den")
        num = dpool.tile([128, FLUSH, 2], FP, name="num", tag="num")
        na = dpool.tile([128, FLUSH, 2], FP, name="na", tag="na")
        rcp = dpool.tile([128, FLUSH, 2], FP, name="rcp", tag="rcp")
        dn, nm, nA, rc = den[:, 0:nf, :], num[:, 0:nf, :], na[:, 0:nf, :], rcp[:, 0:nf, :]
        nc.vector.tensor_add(dn, r[:, :, :, 0, 0], r[:, :, :, 1, 0])
        nc.vector.tensor_add(nA, r[:, :, :, 0, 1], r[:, :, :, 0, 2])
        nc.vector.tensor_add(nm, r[:, :, :, 1, 1], r[:, :, :, 1, 2])
        nc.vector.tensor_add(nm, nm, nA)
        nc.vector.reciprocal_approx_fast(out=rc, in_=dn)
        nc.vector.tensor_mul(nm, nm, rc)
        nc.gpsimd.dma_start(
            out=od[:, p0 * 2:p0 * 2 + nf * 2],
            in_=nm.rearrange("p i h -> p (i h)"),
        )
        fs["tile"] = None
        fs["count"] = 0

    def emit_reduces(pend):
        e, members = pend
        for (s, g) in members:
            if fs["count"] == 0:
                fs["tile"] = ps_res.tile([128, 512], FP, name="res", tag="res")
                fs["first_g"] = g
            col0 = fs["count"] * 12
            for h in (0, 1):
                for t in (0, 1):
                    nc.tensor.matmul(
                        fs["tile"][:, col0 + h * 6 + t * 3: col0 + h * 6 + t * 3 + 3],
                        lhsT=e[:][:, s * 512 + t * 256 + h * 128: s * 512 + t * 256 + (h + 1) * 128],
                        rhs=vt[:, g, t, :].bitcast(BF16),
                        start=True, stop=True,
                        tile_position=(0, 0),
                    )
            fs["count"] += 1
            if fs["count"] == FLUSH or g == PAIRS - 1:
                division_flush()

    # --- main loop -----------------------------------------------------------
    qt0, kt0 = load_chunk(0)
    vt = vpool.tile([128, 512, 2, 3], U16, name="vt", tag="vt")
    nc.gpsimd.dma_start(out=vt[:].rearrange("p g t c -> p (g t c)"), in_=vd[:])

    qt = kt = None
    cur_stage = None
    members = []
    pending = None
    ci = -1
    for g in range(PAIRS):
        if ci + 1 < NCH and g == CHUNK_STARTS[ci + 1]:
            ci += 1
            qt, kt = (qt0, kt0) if ci == 0 else load_chunk(ci)
        j = g - CHUNK_STARTS[ci]
        strip, slot = j % STRIPS, j // STRIPS
        s = g % GROUP
        if s == 0:
            cur_stage = ps_stage.tile([128, GROUP * 512], FP, name="st", tag="st")
            members = []
        nc.tensor.matmul(
            cur_stage[:, s * 512:(s + 1) * 512],
            lhsT=kt[32 * strip:32 * strip + 2, slot * 128:(slot + 1) * 128],
            rhs=qt[32 * strip:32 * strip + 2, slot * 512:(slot + 1) * 512],
            start=True, stop=True,
            tile_position=(32 * strip, 0),
        )
        members.append((s, g))
        if s == GROUP - 1 or g == PAIRS - 1:
            e = epool.tile([128, GROUP * 512], BF16, name="e", tag="e")
            width = len(members) * 512
            nc.scalar.activation(e[:][:, 0:width], cur_stage[:][:, 0:width], AF.Exp)
            if pending is not None:
                emit_reduces(pending)
            pending = (e, members)
    emit_reduces(pending)
    assert fs["count"] == 0


_CACHE: dict = {}


def _get_nc(reps: int = 1) -> bass.Bass:
    if reps not in _CACHE:
        _CACHE[reps] = build_kernel_module(reps)
    return _CACHE[reps]


def _to_bf16_bits(x: np.ndarray) -> np.ndarray:
    """fp32 -> bf16 bit pattern (round-to-nearest-even), as uint16."""
    u = x.astype(np.float32).view(np.uint32)
    rounded = u + 0x7FFF + ((u >> 16) & 1)
    return (rounded >> 16).astype(np.uint16)


def _bf16_to_f32(bits: np.ndarray) -> np.ndarray:
    return (bits.astype(np.uint32) << 16).view(np.float32)


def make_in_maps(query, key, value, in_proj_w, in_proj_b, out_proj_w, out_proj_b):
    q = np.asarray(query, dtype=np.float32).reshape(L, N)
    k = np.asarray(key, dtype=np.float32).reshape(L, N)
    v = np.asarray(value, dtype=np.float32).reshape(L, N)
    wq, wk, wv = [float(x) for x in np.asarray(in_proj_w, dtype=np.float32).reshape(3)]
    bq, bk, bv = [float(x) for x in np.asarray(in_proj_b, dtype=np.float32).reshape(3)]
    wo = float(np.asarray(out_proj_w, dtype=np.float32).reshape(1)[0])
    bo = float(np.asarray(out_proj_b, dtype=np.float32).reshape(1)[0])

    qp = q * np.float32(wq) + np.float32(bq)
    kp = k * np.float32(wk) + np.float32(bk)
    vp = v * (np.float32(wo) * np.float32(wv))
    out_bias = float(np.float32(wo) * np.float32(bv) + np.float32(bo))

    one_bits = np.uint16(0x3F80)  # bf16 1.0
    in_maps = []
    for c in range(NCORES):
        sl = slice(c * LS, (c + 1) * LS)
        # [g, 256] with g = b*32 + n
        Q = np.ascontiguousarray(qp[sl].reshape(BPC, BS, N).transpose(0, 2, 1)).reshape(PAIRS, BS)
        K = np.ascontiguousarray(kp[sl].reshape(BPC, BS, N).transpose(0, 2, 1)).reshape(PAIRS, BS)
        V = np.ascontiguousarray(vp[sl].reshape(BPC, BS, N).transpose(0, 2, 1)).reshape(PAIRS, BS)

        # q rhs: rows [ci*2*STRIPS + s*2 + r], cols [slot*512 + half*256 + :256]
        qd_np = np.zeros((NCH * 2 * STRIPS, 8192), np.float32)
        kd_np = np.zeros((NCH * 2 * STRIPS, 2048), np.float32)
        for ci, (start, size) in enumerate(zip(CHUNK_STARTS, CHUNK_SIZES)):
            ns = size // STRIPS
            Qv = Q[start:start + size].reshape(ns, STRIPS, BS)      # [slot, s, :]
            q4 = np.zeros((STRIPS, 2, ns, 2, BS), np.float32)
            q4[:, 0, :, 0, :] = Qv.transpose(1, 0, 2)
            q4[:, 1, :, 1, :] = Qv.transpose(1, 0, 2)
            qd_np[ci * 2 * STRIPS:(ci + 1) * 2 * STRIPS, 0:ns * 512] = \
                q4.reshape(2 * STRIPS, ns * 512)
            Kv = K[start:start + size].reshape(ns, STRIPS, 2, 128)  # [slot, s, r, :]
            kd_np[ci * 2 * STRIPS:(ci + 1) * 2 * STRIPS, 0:ns * 128] = \
                np.ascontiguousarray(Kv.transpose(1, 2, 0, 3)).reshape(2 * STRIPS, ns * 128)

        # v reduce rhs: [p, g, t, (1, hi, lo)] -> [128, 3072] bf16 bits
        vhi_bits = _to_bf16_bits(V)
        vlo_bits = _to_bf16_bits(V - _bf16_to_f32(vhi_bits))
        vr = np.empty((128, PAIRS, 2, 3), np.uint16)
        vr[:, :, :, 0] = one_bits
        vr[:, :, :, 1] = vhi_bits.reshape(PAIRS, 2, 128).transpose(2, 0, 1)
        vr[:, :, :, 2] = vlo_bits.reshape(PAIRS, 2, 128).transpose(2, 0, 1)

        in_maps.append({
            "qd": np.ascontiguousarray(qd_np),
            "kd": kd_np,
            "vd": np.ascontiguousarray(vr.reshape(128, 3072)),
        })
    return in_maps, out_bias


def run(in_maps, **kwargs):
    return run_bass_kernel_spmd(_get_nc(), in_maps, list(range(NCORES)), **kwargs)


def assemble(results, out_bias) -> np.ndarray:
    shards = []
    for c in range(NCORES):
        od = np.asarray(results[c]["od"], dtype=np.float32)  # [128, 1024]
        arr = od.reshape(128, BPC, N, 2)                     # [qh, b, n, h]
        shards.append(arr.transpose(1, 3, 0, 2).reshape(LS, N))
    out = np.concatenate(shards, axis=0) + np.float32(out_bias)
    return np.ascontiguousarray(out).reshape(L, N, 1)


def kernel(query, key, value, in_proj_w, in_proj_b, out_proj_w, out_proj_b):
    in_maps, out_bias = make_in_maps(
        query, key, value, in_proj_w, in_proj_b, out_proj_w, out_proj_b
    )
    res = run(in_maps)
    return assemble(res.results, out_bias)


# revision 21
# speedup vs baseline: 1.4349x; 1.0096x over previous
"""Trainium2 Bass kernel for nn_BlockCrossAttn (block-diagonal attention, E=H=1).

Math per (block b, batch n) pair (256-long vectors q', k', v' of the block):
    q' = wq*Q + bq ; k' = wk*K + bk      (folded on host)
    soft[q,k] = softmax_k(q'[q] * k'[k])
    out[q] = sum_k soft[q,k] * (wo*wv*V[k]) + (wo*bv + bo)   (bias added on host)
No max-subtraction: |scores| <= ~27 worst case, exp is safe in fp32.

Sharding: 128 blocks of 256 rows; 16 blocks per core across 8 cores
(fully independent, no collectives).

Per-core device pipeline (512 pairs):
  - Scores: per pair ONE f32r matmul: lhsT [2, 128] = both 128-long k-halves
    (one weight load covers the pair), rhs [2, 512] = zero-padded q rows
    ([q|0] on row 0, [0|q] on row 1) -> S^T[k-in-half, (t, q)] = [128, 512]
    in PSUM.  Pairs rotate tile_position rows {0,32,64,96} so weight loads
    overlap matmuls of other strips.
  - ScalarE exp over [128, 1536] PSUM spans (3 pairs/group) -> E bf16 SBUF.
  - Reduce: lhsT = E 128-col slices (bf16 -> fast weight load), rhs =
    [ones, v_hi, v_lo] 3 columns -> PSUM [q-half 128, 3] = (den, num_hi,
    num_lo) per (pair, q-half, k-half), all partition-ALIGNED per q.
    Every matmul start=stop=True (no accumulation hazards).
  - VectorE epilogue straight on the result bank: den = t0+t1, num = 4-way
    add, reciprocal_approx_fast, multiply; contiguous DMA to a [128, 1024]
    q-major output that the host unscrambles.

All weight scalars are folded into the inputs on the host, so the compiled
module is weight-independent (compiled once, cached).
"""

from contextlib import ExitStack

import numpy as np

import concourse.bacc as bacc
import concourse.bass as bass
import concourse.tile as tile
from concourse import mybir
from concourse.bass_utils import run_bass_kernel_spmd

FP = mybir.dt.float32
F32R = mybir.dt.float32r
BF16 = mybir.dt.bfloat16
U16 = mybir.dt.uint16
AF = mybir.ActivationFunctionType

L = 32768          # sequence length
N = 32             # batch
BS = 256           # block size
NB = L // BS       # 128 blocks
NCORES = 8
BPC = NB // NCORES  # 16 blocks per core
LS = BPC * BS       # 4096 rows per core shard
PAIRS = BPC * N     # 512 (block, batch) pairs per core

GROUP = 3           # pairs per exp staging group (3 PSUM banks)
STRIPS = 4          # tile_position row strips used for outer matmuls
CHUNK = 16 * STRIPS  # pairs per full q/k input chunk
# First chunk split small so the first matmuls start ASAP after launch.
CHUNK_SIZES = [8, 24, 32] + [CHUNK] * ((PAIRS - CHUNK) // CHUNK)
assert sum(CHUNK_SIZES) == PAIRS and all(c % STRIPS == 0 for c in CHUNK_SIZES)
CHUNK_STARTS = [sum(CHUNK_SIZES[:i]) for i in range(len(CHUNK_SIZES))]
NCH = len(CHUNK_SIZES)
FLUSH = 42          # pairs per result-bank flush (42*12 = 504 <= 512 cols)


def build_kernel_module(reps: int = 1) -> bass.Bass:
    nc = bacc.Bacc("TRN2", target_bir_lowering=False, debug=False, num_devices=NCORES)
    # q rhs blocks: [ch*2*STRIPS + s*2 + r, slot*512 + half*256 + :256]
    qd = nc.declare_dram_parameter("qd", [NCH * 2 * STRIPS, 8192], F32R, isOutput=False)
    # k lhsT blocks: [ch*2*STRIPS + s*2 + r, slot*128 + :128]
    kd = nc.declare_dram_parameter("kd", [NCH * 2 * STRIPS, 2048], F32R, isOutput=False)
    # v reduce rhs: [p, g*6 + t*3 + (1.0, v_hi, v_lo)] as bf16 bits
    vd = nc.declare_dram_parameter("vd", [128, 3072], U16, isOutput=False)
    # out: [q-in-half, pair*2 + h]
    od = nc.declare_dram_parameter("od", [128, 1024], FP, isOutput=True)

    with tile.TileContext(nc) as tc:
        with ExitStack() as ctx:
            if reps == 1:
                _emit(ctx, tc, qd, kd, vd, od)
            else:
                with tc.For_i(0, reps, 1):
                    _emit(ctx, tc, qd, kd, vd, od)
    nc.compile()
    return nc


def _emit(ctx, tc, qd, kd, vd, od):
    nc = tc.nc

    qpool = ctx.enter_context(tc.tile_pool(name="qpool", bufs=3))
    kpool = ctx.enter_context(tc.tile_pool(name="kpool", bufs=3))
    vpool = ctx.enter_context(tc.tile_pool(name="vpool", bufs=1))
    epool = ctx.enter_context(tc.tile_pool(name="epool", bufs=4))
    dpool = ctx.enter_context(tc.tile_pool(name="dpool", bufs=2))
    ps_stage = ctx.enter_context(tc.tile_pool(name="ps_stage", bufs=2, space="PSUM"))
    ps_res = ctx.enter_context(tc.tile_pool(name="ps_res", bufs=2, space="PSUM"))

    # Warm the exp table set (~2.7us) while the first DMAs run.
    warm = vpool.tile([1, 8], FP, name="warm", tag="warm")
    nc.vector.memset(warm[:], 0.0)
    nc.scalar.activation(warm[:], warm[:], AF.Exp)

    def load_chunk(ci):
        size = CHUNK_SIZES[ci]
        wq = (size // STRIPS) * 512
        wk = (size // STRIPS) * 128
        qt = qpool.tile([128, 8192], F32R, name="qt", tag="qt")
        kt = kpool.tile([128, 2048], F32R, name="kt", tag="kt")
        for s in range(STRIPS):
            row = ci * 2 * STRIPS + s * 2
            nc.sync.dma_start(out=qt[32 * s:32 * s + 2, 0:wq], in_=qd[row:row + 2, 0:wq])
            nc.sync.dma_start(out=kt[32 * s:32 * s + 2, 0:wk], in_=kd[row:row + 2, 0:wk])
        return qt, kt

    # --- result-bank flush: softmax division epilogue ------------------------
    fs = {"tile": None, "count": 0, "first_g": 0}

    def division_flush():
        nf = fs["count"]
        p0 = fs["first_g"]
        sb = dpool.tile([128, FLUSH * 12], FP, name="sb", tag="sb")
        nc.vector.tensor_copy(sb[:, 0:nf * 12], fs["tile"][:, 0:nf * 12])
        r = sb[:, 0:nf * 12].rearrange("p (i h t c) -> p i h t c", h=2, t=2, c=3)
        den = dpool.tile([128, FLUSH, 2], FP, name="den", tag="den")
        num = dpool.tile([128, FLUSH, 2], FP, name="num", tag="num")
        na = dpool.tile([128, FLUSH, 2], FP, name="na", tag="na")
        rcp = dpool.tile([128, FLUSH, 2], FP, name="rcp", tag="rcp")
        dn, nm, nA, rc = den[:, 0:nf, :], num[:, 0:nf, :], na[:, 0:nf, :], rcp[:, 0:nf, :]
        nc.vector.tensor_add(dn, r[:, :, :, 0, 0], r[:, :, :, 1, 0])
        nc.vector.tensor_add(nA, r[:, :, :, 0, 1], r[:, :, :, 0, 2])
        nc.vector.tensor_add(nm, r[:, :, :, 1, 1], r[:, :, :, 1, 2])
        nc.vector.tensor_add(nm, nm, nA)
        nc.vector.reciprocal_approx_fast(out=rc, in_=dn)
        nc.vector.tensor_mul(nm, nm, rc)
        nc.gpsimd.dma_start(
            out=od[:, p0 * 2:p0 * 2 + nf * 2],
            in_=nm.rearrange("p i h -> p (i h)"),
        )
        fs["tile"] = None
        fs["count"] = 0

    def emit_reduces(pend):
        e, members = pend
        for (s, g) in members:
            if fs["count"] == 0:
                fs["tile"] = ps_res.tile([128, 512], FP, name="res", tag="res")
                fs["first_g"] = g
            col0 = fs["count"] * 12
            for h in (0, 1):
                for t in (0, 1):
                    nc.tensor.matmul(
                        fs["tile"][:, col0 + h * 6 + t * 3: col0 + h * 6 + t * 3 + 3],
                        lhsT=e[:][:, s * 512 + t * 256 + h * 128: s * 512 + t * 256 + (h + 1) * 128],
                        rhs=vt[:, g, t, :].bitcast(BF16),
                        start=True, stop=True,
                        tile_position=(0, 0),
                    )
            fs["count"] += 1
            if fs["count"] == FLUSH or g == PAIRS - 1:
                division_flush()

    # --- main loop -----------------------------------------------------------
    qt0, kt0 = load_chunk(0)
    vt = vpool.tile([128, 512, 2, 3], U16, name="vt", tag="vt")
    nc.gpsimd.dma_start(out=vt[:].rearrange("p g t c -> p (g t c)"), in_=vd[:])

    qt = kt = None
    cur_stage = None
    members = []
    pending = []
    ci = -1
    for g in range(PAIRS):
        if ci + 1 < NCH and g == CHUNK_STARTS[ci + 1]:
            ci += 1
            qt, kt = (qt0, kt0) if ci == 0 else load_chunk(ci)
        j = g - CHUNK_STARTS[ci]
        strip, slot = j % STRIPS, j // STRIPS
        s = g % GROUP
        if s == 0:
            cur_stage = ps_stage.tile([128, GROUP * 512], FP, name="st", tag="st")
            members = []
        nc.tensor.matmul(
            cur_stage[:, s * 512:(s + 1) * 512],
            lhsT=kt[32 * strip:32 * strip + 2, slot * 128:(slot + 1) * 128],
            rhs=qt[32 * strip:32 * strip + 2, slot * 512:(slot + 1) * 512],
            start=True, stop=True,
            tile_position=(32 * strip, 0),
        )
        members.append((s, g))
        if s == GROUP - 1 or g == PAIRS - 1:
            e = epool.tile([128, GROUP * 512], BF16, name="e", tag="e")
            width = len(members) * 512
            nc.scalar.activation(e[:][:, 0:width], cur_stage[:][:, 0:width], AF.Exp)
            pending.append((e, members))
            # Reduces trail TWO groups behind the activation, so their
            # weight loads never wait on a still-running ACT (that wait
            # head-of-line-blocks the next groups' score matmuls).
            if len(pending) > 2:
                emit_reduces(pending.pop(0))
    for pend in pending:
        emit_reduces(pend)
    assert fs["count"] == 0


_CACHE: dict = {}


def _get_nc(reps: int = 1) -> bass.Bass:
    if reps not in _CACHE:
        _CACHE[reps] = build_kernel_module(reps)
    return _CACHE[reps]


def _to_bf16_bits(x: np.ndarray) -> np.ndarray:
    """fp32 -> bf16 bit pattern (round-to-nearest-even), as uint16."""
    u = x.astype(np.float32).view(np.uint32)
    rounded = u + 0x7FFF + ((u >> 16) & 1)
    return (rounded >> 16).astype(np.uint16)


def _bf16_to_f32(bits: np.ndarray) -> np.ndarray:
    return (bits.astype(np.uint32) << 16).view(np.float32)


def make_in_maps(query, key, value, in_proj_w, in_proj_b, out_proj_w, out_proj_b):
    q = np.asarray(query, dtype=np.float32).reshape(L, N)
    k = np.asarray(key, dtype=np.float32).reshape(L, N)
    v = np.asarray(value, dtype=np.float32).reshape(L, N)
    wq, wk, wv = [float(x) for x in np.asarray(in_proj_w, dtype=np.float32).reshape(3)]
    bq, bk, bv = [float(x) for x in np.asarray(in_proj_b, dtype=np.float32).reshape(3)]
    wo = float(np.asarray(out_proj_w, dtype=np.float32).reshape(1)[0])
    bo = float(np.asarray(out_proj_b, dtype=np.float32).reshape(1)[0])

    qp = q * np.float32(wq) + np.float32(bq)
    kp = k * np.float32(wk) + np.float32(bk)
    vp = v * (np.float32(wo) * np.float32(wv))
    out_bias = float(np.float32(wo) * np.float32(bv) + np.float32(bo))

    one_bits = np.uint16(0x3F80)  # bf16 1.0
    in_maps = []
    for c in range(NCORES):
        sl = slice(c * LS, (c + 1) * LS)
        # [g, 256] with g = b*32 + n
        Q = np.ascontiguousarray(qp[sl].reshape(BPC, BS, N).transpose(0, 2, 1)).reshape(PAIRS, BS)
        K = np.ascontiguousarray(kp[sl].reshape(BPC, BS, N).transpose(0, 2, 1)).reshape(PAIRS, BS)
        V = np.ascontiguousarray(vp[sl].reshape(BPC, BS, N).transpose(0, 2, 1)).reshape(PAIRS, BS)

        # q rhs: rows [ci*2*STRIPS + s*2 + r], cols [slot*512 + half*256 + :256]
        qd_np = np.zeros((NCH * 2 * STRIPS, 8192), np.float32)
        kd_np = np.zeros((NCH * 2 * STRIPS, 2048), np.float32)
        for ci, (start, size) in enumerate(zip(CHUNK_STARTS, CHUNK_SIZES)):
            ns = size // STRIPS
            Qv = Q[start:start + size].reshape(ns, STRIPS, BS)      # [slot, s, :]
            q4 = np.zeros((STRIPS, 2, ns, 2, BS), np.float32)
            q4[:, 0, :, 0, :] = Qv.transpose(1, 0, 2)
            q4[:, 1, :, 1, :] = Qv.transpose(1, 0, 2)
            qd_np[ci * 2 * STRIPS:(ci + 1) * 2 * STRIPS, 0:ns * 512] = \
                q4.reshape(2 * STRIPS, ns * 512)
            Kv = K[start:start + size].reshape(ns, STRIPS, 2, 128)  # [slot, s, r, :]
            kd_np[ci * 2 * STRIPS:(ci + 1) * 2 * STRIPS, 0:ns * 128] = \
                np.ascontiguousarray(Kv.transpose(1, 2, 0, 3)).reshape(2 * STRIPS, ns * 128)

        # v reduce rhs: [p, g, t, (1, hi, lo)] -> [128, 3072] bf16 bits
        vhi_bits = _to_bf16_bits(V)
        vlo_bits = _to_bf16_bits(V - _bf16_to_f32(vhi_bits))
        vr = np.empty((128, PAIRS, 2, 3), np.uint16)
        vr[:, :, :, 0] = one_bits
        vr[:, :, :, 1] = vhi_bits.reshape(PAIRS, 2, 128).transpose(2, 0, 1)
        vr[:, :, :, 2] = vlo_bits.reshape(PAIRS, 2, 128).transpose(2, 0, 1)

        in_maps.append({
            "qd": np.ascontiguousarray(qd_np),
            "kd": kd_np,
            "vd": np.ascontiguousarray(vr.reshape(128, 3072)),
        })
    return in_maps, out_bias


def run(in_maps, **kwargs):
    return run_bass_kernel_spmd(_get_nc(), in_maps, list(range(NCORES)), **kwargs)


def assemble(results, out_bias) -> np.ndarray:
    shards = []
    for c in range(NCORES):
        od = np.asarray(results[c]["od"], dtype=np.float32)  # [128, 1024]
        arr = od.reshape(128, BPC, N, 2)                     # [qh, b, n, h]
        shards.append(arr.transpose(1, 3, 0, 2).reshape(LS, N))
    out = np.concatenate(shards, axis=0) + np.float32(out_bias)
    return np.ascontiguousarray(out).reshape(L, N, 1)


def kernel(query, key, value, in_proj_w, in_proj_b, out_proj_w, out_proj_b):
    in_maps, out_bias = make_in_maps(
        query, key, value, in_proj_w, in_proj_b, out_proj_w, out_proj_b
    )
    res = run(in_maps)
    return assemble(res.results, out_bias)
